# revision 8
# baseline (speedup 1.0000x reference)
"""Multi-head causal self-attention (V=Q variant) on 8 Trainium2 cores, v2.

Sharding: batch (2) x head-group (4 groups of 4 heads). Each core computes
full-sequence attention for its 4 heads (2 head-pairs) of one batch element
plus its slice of the output projection; the host sums 4 partials per batch
and adds b0.

v2 redesign vs v1 (145.4us -> 108.8us in the cost model):
- K projection and scores run in fp8e4m3 DoubleRow (0.5 cycles/row in the
  model). Scores keep a 64-deep contraction at the DoubleRow rate via a
  stride-0 broadcast second k-tile (doubling the result); Wk is
  host-prescaled by 128 to keep fp8 normals; the 1/128 sits in the K
  eviction and the 2x and 1/sqrt(DK) in the exp scale (0.0625).
- AV is computed reversed: att[q,65] = probs_chunk^T @ [V|1], filling all
  128 output partitions per pass (the forward form filled only 65). The
  softmax denominator still falls out of the ones column. Each att PSUM
  bank holds 4 AV slots under ONE accumulation group (start zeroes the
  whole 2KB zero region; only the first/last matmuls set start/stop).
  Normalization = DVE reciprocal + per-partition scalar multiply once the
  group closes; attn^T for the output projection is rebuilt with batched
  DMA XBAR transposes (as are the V' tiles from QT).
- exp is one [128,1024] activation per score tile: the B head's scores
  land at column 512 so diagonal tiles stay contiguous with A's [off:512].
  Scores run one step ahead of exps (lead-1) and AV three steps behind, so
  ACT (the bottleneck engine, ~77us busy) is fed with minimal stalls.
- GPSIMD cannot touch PSUM on real hardware: all PSUM evictions live on
  DVE (ACT helps in the post-exp endgame); Pool does the SBUF-only work
  (causal masks, V' ones columns, warmup memsets).
- PSUM: scores 2x[128,1024] double-buffered (4 banks) + att 2 banks +
  proj/outproj shared pool (2 banks) = all 8 banks.
"""

import ml_dtypes
import numpy as np

import concourse.bacc as bacc
import concourse.mybir as mybir
from concourse.tile import TileContext, add_dep_helper

P = 128
S = 2048
D = 1024
HD = 256
DK = 64
NQ = 4  # q blocks of 512
NKD = 8  # D chunks of 128
F32 = mybir.dt.float32
BF16 = mybir.dt.bfloat16
FP8 = mybir.dt.float8e4
EXP = mybir.ActivationFunctionType.Exp
DR = mybir.MatmulPerfMode.DoubleRow
MUL = mybir.AluOpType.mult
ADD = mybir.AluOpType.add

_CACHED_NC = None


def build_nc():
    nc = bacc.Bacc("TRN2", target_bir_lowering=False, debug=False, num_devices=8)
    xT = nc.declare_dram_parameter("xT", [D, S], BF16, isOutput=False)
    x8 = nc.declare_dram_parameter("x8", [D, S], FP8, isOutput=False)
    Wq = nc.declare_dram_parameter("Wq", [P, NKD, HD], BF16, isOutput=False)
    Wk8 = nc.declare_dram_parameter("Wk8", [P, NKD, HD], FP8, isOutput=False)
    bqt = nc.declare_dram_parameter("bqt", [P, 2], F32, isOutput=False)
    bkt = nc.declare_dram_parameter("bkt", [P, 2], F32, isOutput=False)
    W0 = nc.declare_dram_parameter("W0", [HD, D], BF16, isOutput=False)
    out = nc.declare_dram_parameter("out", [S, D], BF16, isOutput=True)

    with TileContext(nc) as tc:
        with (
            tc.tile_pool(name="const", bufs=1) as const,
            tc.tile_pool(name="xt", bufs=8) as xtp,
            tc.tile_pool(name="x8p", bufs=4) as x8p,
            tc.tile_pool(name="pp", bufs=24) as ppool,
            tc.tile_pool(name="an", bufs=4) as anp,
            tc.tile_pool(name="tst", bufs=3) as tstg,
            tc.tile_pool(name="rc", bufs=4) as rcp,
            tc.tile_pool(name="ost", bufs=3) as ostp,
            tc.tile_pool(name="sps", bufs=2, space="PSUM") as sps,
            tc.tile_pool(name="aps", bufs=2, space="PSUM") as aps,
            tc.tile_pool(name="mm", bufs=2, space="PSUM") as mmp,
        ):
            # ---- PE p-state warmup first: dummy matmuls keep the PE busy
            # through the initial DMA wait so real matmuls start at full clock
            wa = const.tile([P, 256], BF16, name="wa")
            nc.gpsimd.memset(wa[:], 0.0)
            for _ in range(14):
                wps = mmp.tile([P, 256], F32, name="ps")
                nc.tensor.matmul(wps[:], lhsT=wa[:, 0:128], rhs=wa[:], start=True, stop=True)

            # ---- constants / static tiles
            # triangular keep-mask [128,128]: 1.0 where q >= kv
            tri = const.tile([P, P], BF16, name="tri")
            nc.gpsimd.memset(tri[:], 1.0)
            nc.gpsimd.affine_select(
                out=tri[:],
                in_=tri[:],
                compare_op=mybir.AluOpType.is_ge,
                fill=0.0,
                base=0,
                pattern=[[1, P]],
                channel_multiplier=-1,
            )
            # ACT exp-table warmup while DMAs run
            warm = const.tile([P, 8], F32, name="warm")
            nc.gpsimd.memset(warm[:], 0.0)
            nc.scalar.activation(out=warm[:], in_=warm[:], func=EXP)

            bq_sb = const.tile([P, 2], F32)
            bk_sb = const.tile([P, 2], F32)
            w0_sb = [const.tile([P, D], BF16, name=f"w0_{p}") for p in range(2)]
            # projections: QT (bf16, feeds V' transposes), QT8/KT8 (fp8;
            # scores use a stride-0 broadcast second k-tile, doubling the
            # result -- absorbed by the exp scale)
            QT = [
                [const.tile([P, 512], BF16, name=f"qt{mi}_{ni}") for ni in range(NQ)]
                for mi in range(2)
            ]
            QT8 = [
                [const.tile([P, 1, 512], FP8, name=f"qt8{mi}_{ni}") for ni in range(NQ)]
                for mi in range(2)
            ]
            KT8 = [
                [const.tile([P, 1, 512], FP8, name=f"kt8{mi}_{ni}") for ni in range(NQ)]
                for mi in range(2)
            ]
            vt = {}
            for pair in range(2):
                for i in range(16):
                    vt[(pair, i)] = const.tile([P, 2, 65], BF16, name=f"vt{pair}_{i}")
            attnT = {}  # (pair, m-pair) -> [P, 2, P]; [:, m%2, :] = attn^T(m)
            for pair in range(2):
                for mp in range(8):
                    attnT[(pair, mp)] = const.tile([P, 2, P], BF16, name=f"at{pair}_{mp}")

            # ones column of every V' tile (Pool, pair-0 first); written once
            for key in vt:
                nc.gpsimd.memset(vt[key][:, :, 64:65], 1.0)

            # ---- DMAs, ordered so the first Q/K sweeps can start early.
            # x arrives in 512-column (q-chunk) slabs, k-major inside.
            xq = [[None, None] for _ in range(NQ)]  # [qc][lohi] -> [P, 4, 512]
            x8q = [None] * NQ  # [qc] -> [P, 8, 512] fp8
            wq_big = const.tile([P, NKD, HD], BF16, name="wqb")
            wk8_big = const.tile([P, NKD, HD], FP8, name="wkb")

            def dma_x16(qc):
                cs = slice(qc * 512, (qc + 1) * 512)
                for lohi in range(2):
                    t = xtp.tile([P, 4, 512], BF16, name="xq")
                    nc.sync.dma_start(
                        out=t[:],
                        in_=xT[lohi * 512 : (lohi + 1) * 512, cs].rearrange(
                            "(k p) s -> p k s", p=P
                        ),
                    )
                    xq[qc][lohi] = t

            def dma_x8(qc):
                cs = slice(qc * 512, (qc + 1) * 512)
                t8 = x8p.tile([P, 8, 512], FP8, name="x8q")
                nc.sync.dma_start(
                    out=t8[:], in_=x8[:, cs].rearrange("(k p) s -> p k s", p=P)
                )
                x8q[qc] = t8

            nc.sync.dma_start(out=wq_big[:], in_=Wq[:, :, :])
            dma_x16(0)
            nc.sync.dma_start(out=bq_sb[:], in_=bqt[:, :])
            nc.sync.dma_start(out=bk_sb[:], in_=bkt[:, :])
            nc.sync.dma_start(out=wk8_big[:], in_=Wk8[:, :, :])
            dma_x8(0)
            for qc in range(1, NQ):
                dma_x16(qc)
                dma_x8(qc)
            for p in range(2):
                nc.sync.dma_start(
                    out=w0_sb[p][:],
                    in_=W0[p * P : (p + 1) * P, :],
                )

            # ---- sweep emitters (as drip-able item lists)
            def q_sweep_items(ni, mi):
                ps = mmp.tile([P, 512], F32, name="ps")

                def mk(k):
                    def go():
                        nc.tensor.matmul(
                            ps[:],
                            lhsT=wq_big[:, k, mi * P : (mi + 1) * P],
                            rhs=xq[ni][k // 4][:, k % 4, :],
                            start=(k == 0),
                            stop=(k == NKD - 1),
                        )

                    return go

                def evict():
                    # two direct evictions: fp8 for scores (critical path),
                    # bf16 for the V' transposes
                    nc.vector.tensor_scalar_add(
                        QT8[mi][ni][:, 0, :], ps[:], bq_sb[:, mi : mi + 1]
                    )
                    nc.vector.tensor_scalar_add(
                        QT[mi][ni][:, :], ps[:], bq_sb[:, mi : mi + 1]
                    )

                return [(2, mk(k)) for k in range(NKD)] + [(1, evict)]

            def k_sweep_items(ni, mi):
                ps = mmp.tile([P, 512], F32, name="ps")

                def mk(kp):
                    def go():
                        nc.tensor.matmul(
                            ps[:],
                            lhsT=wk8_big[:, 2 * kp : 2 * kp + 2, mi * P : (mi + 1) * P],
                            rhs=x8q[ni][:, 2 * kp : 2 * kp + 2, :],
                            start=(kp == 0),
                            stop=(kp == 3),
                            perf_mode=DR,
                        )

                    return go

                def evict():
                    nc.vector.tensor_scalar(
                        out=KT8[mi][ni][:, 0, :],
                        in0=ps[:],
                        scalar1=1.0 / 128.0,
                        scalar2=bk_sb[:, mi : mi + 1],
                        op0=MUL,
                        op1=ADD,
                    )

                return [(2, mk(kp)) for kp in range(4)] + [(1, evict)]

            def vT_items(pair, ni):
                """one batched DMA transpose per QT tile covers 4 V' chunks"""
                state = {}

                def tp_go():
                    state["tp"] = tstg.tile([P, 4, P], BF16, name="ts")
                    nc.sync.dma_start_transpose(out=state["tp"][:], in_=QT[pair][ni][:, :])

                def cp(c):
                    def go():
                        v = vt[(pair, 4 * ni + c)]
                        src_ap = state["tp"][:, c, :].rearrange("p (h d) -> p h d", h=2)
                        nc.vector.tensor_copy(v[:, :, 0:64], src_ap)

                    return go

                return [(2, tp_go)] + [(1, cp(c)) for c in range(4)]

            # ---- drip queue: background emit-thunks (sweeps, V'T, outproj)
            # items may carry a min step number (global exp-step counter) so
            # work that waits on a fresh DMA-transpose isn't popped while its
            # input is still in flight (it would stall the in-order PE queue)
            bg = []
            stepno = [0]

            def drip(budget):
                i2 = 0
                while i2 < len(bg) and budget > 0:
                    item = bg[i2]
                    if len(item) == 3 and item[2] > stepno[0]:
                        i2 += 1
                        continue
                    bg.pop(i2)
                    item[1]()
                    budget -= item[0]

            def emit_outproj(m, endgame=False):
                # endgame (post-last-exp): evictions alternate ACT/DVE (ACT is
                # idle by then) and the out DMA goes per-half to start earlier
                state = {}

                def half(n):
                    ps = mmp.tile([P, 512], F32, name="ps")
                    for p_ in range(2):
                        nc.tensor.matmul(
                            ps[:],
                            lhsT=attnT[(p_, m // 2)][:, m % 2, :],
                            rhs=w0_sb[p_][:, n * 512 : (n + 1) * 512],
                            start=(p_ == 0),
                            stop=(p_ == 1),
                        )
                    dst = state["ot"][:, n * 512 : (n + 1) * 512]
                    if endgame and n == 0:
                        nc.scalar.copy(dst, ps[:])
                    else:
                        nc.vector.tensor_copy(dst, ps[:])

                def go0():
                    state["ot"] = ostp.tile([P, D], BF16, name="ot")
                    half(0)
                    if endgame:
                        nc.sync.dma_start(
                            out=out[m * P : (m + 1) * P, 0:512],
                            in_=state["ot"][:, 0:512],
                        )

                def go1():
                    half(1)
                    if endgame:
                        nc.sync.dma_start(
                            out=out[m * P : (m + 1) * P, 512:1024],
                            in_=state["ot"][:, 512:1024],
                        )
                    else:
                        nc.sync.dma_start(
                            out=out[m * P : (m + 1) * P, :], in_=state["ot"][:]
                        )

                return [(2, go0), (2, go1)]

            # ---- attention
            def S_mm(pair, j, i):
                """score matmuls for tile (j, i): S^T doubled via the stride-0
                second k-tile; the 2x and 1/sqrt(DK) sit in the exp scale.
                A lands at [off:512], B at [512:512+w] so one exp covers both."""
                off = max(0, i * P - j * 512)
                w = 512 - off
                kc = slice((i % 4) * P, (i % 4 + 1) * P)
                sAB = sps.tile([P, 1024], F32, name="sab")
                qs = slice(off, 512)
                for h in range(2):
                    hs = slice(h * 64, h * 64 + 64)
                    dst = sAB[:, off:512] if h == 0 else sAB[:, 512 : 512 + w]
                    nc.tensor.matmul(
                        dst,
                        lhsT=KT8[pair][i // 4][hs, :, kc].broadcast_to([64, 2, P]),
                        rhs=QT8[pair][j][hs, :, qs].broadcast_to([64, 2, w]),
                        perf_mode=DR,
                    )
                return sAB

            def S_exp(pair, j, i, sAB):
                """one exp (+ causal masks) for tile (j, i); returns probs."""
                off = max(0, i * P - j * 512)
                w = 512 - off
                pAB = ppool.tile([P, 1024], BF16, name="pab")
                nc.scalar.activation(
                    out=pAB[:, off : 512 + w],
                    in_=sAB[:, off : 512 + w],
                    func=EXP,
                    scale=0.0625,
                )
                if i >= 4 * j:  # diagonal tile: mask the leading 128-col block
                    nc.gpsimd.tensor_mul(
                        pAB[:, off : off + P], pAB[:, off : off + P], tri[:]
                    )
                    nc.gpsimd.tensor_mul(
                        pAB[:, 512 : 512 + P], pAB[:, 512 : 512 + P], tri[:]
                    )
                return pAB

            def av_mm(pair, att, s, m, j, i, pAB):
                # each att bank holds one accumulation GROUP spanning both m
                # slots: start only zeroes once (it clears the whole 2KB zero
                # region), stop only on the very last write to the bank
                cm = (m - 4 * j) * P
                off = max(0, i * P - j * 512)
                last = None
                for h in range(2):
                    lo = cm if h == 0 else 512 + cm - off
                    base = (2 * s + h) * 65
                    last = nc.tensor.matmul(
                        att[:, base : base + 65],
                        lhsT=pAB[:, lo : lo + P],
                        rhs=vt[(pair, i)][:, h, :],
                        start=(i == 0 and s == 0 and h == 0),
                        stop=(i == m and s == 1 and h == 1),
                    )
                return last

            def normalize(pair, att, s, m, an, dep=None):
                rc = rcp.tile([P, 2], F32, name="rc")
                for h in range(2):
                    base = (2 * s + h) * 65
                    r = nc.vector.reciprocal(
                        rc[:, h : h + 1], att[:, base + 64 : base + 65]
                    )
                    if dep is not None and h == 0:
                        # slot-0 values are final, but the bank's accumulation
                        # group only closes at the slot-1 stop matmul; DVE is
                        # in-order so one dep covers the whole normalize
                        add_dep_helper(r.ins, dep.ins, sync=True,
                                       reason="att group close")
                    nc.vector.tensor_scalar(
                        out=an[:, 128 * s + h * 64 : 128 * s + (h + 1) * 64],
                        in0=att[:, base : base + 64],
                        scalar1=rc[:, h : h + 1],
                        scalar2=None,
                        op0=MUL,
                    )

            def av_step(j, ms, att, ip, probs, op, pair=None):
                raise NotImplementedError

            def emit_pair(pair, jorder, budget):
                def av_step(j, ms, att, ip, probs, op):
                    for m in ms:
                        if m < ip:
                            continue
                        t, s = att[m]
                        stop = av_mm(pair, t, s, m, j, ip, probs[ip])
                        if ip == m and s == 1:
                            # group closed: normalize both slots of this bank,
                            # then one batched transpose covers the m-pair
                            an = anp.tile([P, 256], BF16, name="an")
                            normalize(pair, t, 0, m - 1, an, dep=stop)
                            normalize(pair, t, 1, m, an)
                            nc.sync.dma_start_transpose(
                                out=attnT[(pair, m // 2)][:], in_=an[:]
                            )
                            if pair == 1:
                                op(m - 1)
                                op(m)

                # scores run one step ahead of exps (lead-1) so the exp's
                # input semaphore has fired long before ACT gets there
                seq = [(j, i) for j in jorder for i in range(4 * j + 4)]
                sq = {}
                sq[seq[0]] = S_mm(pair, *seq[0])
                idx = 0
                for j in jorder:
                    last = pair == 1 and j == jorder[-1]

                    def op(m, last=last):
                        if last:
                            for _, it in emit_outproj(m, endgame=True):
                                it()
                        else:
                            bg.extend(
                                (c, t, stepno[0] + 3) for c, t in emit_outproj(m)
                            )

                    nsteps = 4 * j + 4
                    probs = {}
                    ms = list(range(4 * j, 4 * j + 4))
                    att = {}  # m -> (tile, slot)
                    pend = []  # i's whose AV is not yet emitted
                    for i in range(nsteps):
                        if idx + 1 < len(seq):
                            sq[seq[idx + 1]] = S_mm(pair, *seq[idx + 1])
                        probs[i] = S_exp(pair, j, i, sq.pop((j, i)))
                        idx += 1
                        stepno[0] += 1
                        pend.append(i)
                        # scale the dripped background work to this step's exp
                        # length so the PE never outruns ACT on short tiles
                        w = 512 - max(0, i * P - j * 512)
                        drip(max(2, budget * (512 + w) // 1024))
                        if i == 0:
                            lo = aps.tile([P, 260], F32, name="att")
                            hi = aps.tile([P, 260], F32, name="att")
                            for s, m in enumerate(ms):
                                att[m] = (lo, s) if s < 2 else (hi, s - 2)
                        if i >= 3:
                            ip = pend.pop(0)
                            av_step(j, ms, att, ip, probs, op)
                    while pend:
                        ip = pend.pop(0)
                        av_step(j, ms, att, ip, probs, op)

            # ---- schedule
            # upfront: first Q/K sweeps + first V' transposes (gate the first
            # score tile), everything else drips
            qs_up = q_sweep_items(0, 0)
            ks_up = k_sweep_items(0, 0)
            for _, it in qs_up[0:4]:
                it()
            for _, it in ks_up[:-1]:
                it()
            for _, it in qs_up[4:8]:
                it()
            ks_up[-1][1]()  # K eviction first (its data lands earlier)
            qs_up[-1][1]()  # then both Q evictions

            for _, it in vT_items(0, 0):
                it()

            order = []
            for ni in (1, 2, 3):
                order += q_sweep_items(ni, 0) + k_sweep_items(ni, 0)
                order += vT_items(0, ni)
            for ni in range(4):
                order += q_sweep_items(ni, 1) + k_sweep_items(ni, 1)
                order += vT_items(1, ni)
            bg.extend(order)

            emit_pair(0, (0, 1, 2, 3), budget=7)
            emit_pair(1, (0, 1, 2, 3), budget=5)
            while bg:
                drip(6)

    nc.compile()
    return nc


def make_in_maps(pos_encode_toks, Wq, bq, Wk, bk, W0, b0):
    x = np.asarray(pos_encode_toks, dtype=np.float32)
    Wq = np.asarray(Wq, dtype=np.float32)
    bq = np.asarray(bq, dtype=np.float32)
    Wk = np.asarray(Wk, dtype=np.float32)
    bk = np.asarray(bk, dtype=np.float32)
    W0 = np.asarray(W0, dtype=np.float32)
    in_maps = []
    for core in range(8):
        b, g = divmod(core, 4)
        hs = slice(g * HD, (g + 1) * HD)
        xt = np.ascontiguousarray(x[b].T)
        in_maps.append(
            {
                "xT": xt.astype(ml_dtypes.bfloat16),
                "x8": xt.astype(ml_dtypes.float8_e4m3),
                "Wq": np.ascontiguousarray(
                    Wq[:, hs].reshape(8, P, HD).transpose(1, 0, 2)
                ).astype(ml_dtypes.bfloat16),
                "Wk8": np.ascontiguousarray(
                    (Wk[:, hs] * 128.0).reshape(8, P, HD).transpose(1, 0, 2)
                ).astype(ml_dtypes.float8_e4m3),
                "bqt": np.ascontiguousarray(bq[hs].reshape(2, P).T),
                "bkt": np.ascontiguousarray(bk[hs].reshape(2, P).T),
                "W0": np.ascontiguousarray(W0[hs, :]).astype(ml_dtypes.bfloat16),
            }
        )
    return in_maps


def assemble(results, b0):
    out = np.zeros((2, S, D), dtype=np.float32)
    for core in range(8):
        b = core // 4
        out[b] += results[core]["out"].astype(np.float32)
    out += np.asarray(b0, dtype=np.float32)
    return out


def kernel(pos_encode_toks, Wq, bq, Wk, bk, W0, b0):
    from concourse.bass_utils import run_bass_kernel_spmd

    global _CACHED_NC
    if _CACHED_NC is None:
        _CACHED_NC = build_nc()
    in_maps = make_in_maps(pos_encode_toks, Wq, bq, Wk, bk, W0, b0)
    res = run_bass_kernel_spmd(_CACHED_NC, in_maps, core_ids=list(range(8)))
    return assemble(res.results, b0)


# revision 9
# speedup vs baseline: 1.0252x; 1.0252x over previous
"""Multi-head causal self-attention (V=Q variant) on 8 Trainium2 cores, v2.

Sharding: batch (2) x head-group (4 groups of 4 heads). Each core computes
full-sequence attention for its 4 heads (2 head-pairs) of one batch element
plus its slice of the output projection; the host sums 4 partials per batch
and adds b0.

v2 redesign vs v1 (145.4us -> 108.8us in the cost model):
- K projection and scores run in fp8e4m3 DoubleRow (0.5 cycles/row in the
  model). Scores keep a 64-deep contraction at the DoubleRow rate via a
  stride-0 broadcast second k-tile (doubling the result); Wk is
  host-prescaled by 128 to keep fp8 normals; the 1/128 sits in the K
  eviction and the 2x and 1/sqrt(DK) in the exp scale (0.0625).
- AV is computed reversed: att[q,65] = probs_chunk^T @ [V|1], filling all
  128 output partitions per pass (the forward form filled only 65). The
  softmax denominator still falls out of the ones column. Each att PSUM
  bank holds 4 AV slots under ONE accumulation group (start zeroes the
  whole 2KB zero region; only the first/last matmuls set start/stop).
  Normalization = DVE reciprocal + per-partition scalar multiply once the
  group closes; attn^T for the output projection is rebuilt with batched
  DMA XBAR transposes (as are the V' tiles from QT).
- exp is one [128,1024] activation per score tile: the B head's scores
  land at column 512 so diagonal tiles stay contiguous with A's [off:512].
  Scores run one step ahead of exps (lead-1) and AV three steps behind, so
  ACT (the bottleneck engine, ~77us busy) is fed with minimal stalls.
- GPSIMD cannot touch PSUM on real hardware: all PSUM evictions live on
  DVE (ACT helps in the post-exp endgame); Pool does the SBUF-only work
  (causal masks, V' ones columns, warmup memsets).
- PSUM: scores 2x[128,1024] double-buffered (4 banks) + att 2 banks +
  proj/outproj shared pool (2 banks) = all 8 banks.
"""

import ml_dtypes
import numpy as np

import concourse.bacc as bacc
import concourse.mybir as mybir
from concourse.tile import TileContext, add_dep_helper

P = 128
S = 2048
D = 1024
HD = 256
DK = 64
NQ = 4  # q blocks of 512
NKD = 8  # D chunks of 128
F32 = mybir.dt.float32
BF16 = mybir.dt.bfloat16
FP8 = mybir.dt.float8e4
EXP = mybir.ActivationFunctionType.Exp
DR = mybir.MatmulPerfMode.DoubleRow
MUL = mybir.AluOpType.mult
ADD = mybir.AluOpType.add

_CACHED_NC = None


def build_nc():
    nc = bacc.Bacc("TRN2", target_bir_lowering=False, debug=False, num_devices=8)
    xT = nc.declare_dram_parameter("xT", [D, S], BF16, isOutput=False)
    x8 = nc.declare_dram_parameter("x8", [D, S], FP8, isOutput=False)
    Wq = nc.declare_dram_parameter("Wq", [P, NKD, HD], BF16, isOutput=False)
    Wk8 = nc.declare_dram_parameter("Wk8", [P, NKD, HD], FP8, isOutput=False)
    bqt = nc.declare_dram_parameter("bqt", [P, 2], F32, isOutput=False)
    bkt = nc.declare_dram_parameter("bkt", [P, 2], F32, isOutput=False)
    W0 = nc.declare_dram_parameter("W0", [HD, D], BF16, isOutput=False)
    out = nc.declare_dram_parameter("out", [S, D], BF16, isOutput=True)

    with TileContext(nc) as tc:
        with (
            tc.tile_pool(name="const", bufs=1) as const,
            tc.tile_pool(name="xt", bufs=8) as xtp,
            tc.tile_pool(name="x8p", bufs=4) as x8p,
            tc.tile_pool(name="pp", bufs=24) as ppool,
            tc.tile_pool(name="an", bufs=4) as anp,
            tc.tile_pool(name="tst", bufs=3) as tstg,
            tc.tile_pool(name="rc", bufs=4) as rcp,
            tc.tile_pool(name="ost", bufs=3) as ostp,
            tc.tile_pool(name="sps", bufs=2, space="PSUM") as sps,
            tc.tile_pool(name="aps", bufs=2, space="PSUM") as aps,
            tc.tile_pool(name="mm", bufs=2, space="PSUM") as mmp,
        ):
            # ---- PE p-state warmup first: dummy matmuls keep the PE busy
            # through the initial DMA wait so real matmuls start at full clock
            wa = const.tile([P, 256], BF16, name="wa")
            nc.gpsimd.memset(wa[:], 0.0)
            for _ in range(14):
                wps = mmp.tile([P, 256], F32, name="ps")
                nc.tensor.matmul(wps[:], lhsT=wa[:, 0:128], rhs=wa[:], start=True, stop=True)

            # ---- constants / static tiles
            identity = const.tile([P, P], BF16)
            nc.gpsimd.memset(identity[:], 0.0)
            nc.gpsimd.affine_select(
                out=identity[:],
                in_=identity[:],
                compare_op=mybir.AluOpType.not_equal,
                fill=1.0,
                base=0,
                pattern=[[-1, P]],
                channel_multiplier=1,
            )
            # triangular keep-mask [128,128]: 1.0 where q >= kv
            tri = const.tile([P, P], BF16, name="tri")
            nc.gpsimd.memset(tri[:], 1.0)
            nc.gpsimd.affine_select(
                out=tri[:],
                in_=tri[:],
                compare_op=mybir.AluOpType.is_ge,
                fill=0.0,
                base=0,
                pattern=[[1, P]],
                channel_multiplier=-1,
            )
            # ACT exp-table warmup while DMAs run
            warm = const.tile([P, 8], F32, name="warm")
            nc.gpsimd.memset(warm[:], 0.0)
            nc.scalar.activation(out=warm[:], in_=warm[:], func=EXP)

            bq_sb = const.tile([P, 2], F32)
            bk_sb = const.tile([P, 2], F32)
            w0_sb = [const.tile([P, D], BF16, name=f"w0_{p}") for p in range(2)]
            # projections: QT (bf16, feeds V' transposes), QT8/KT8 (fp8;
            # scores use a stride-0 broadcast second k-tile, doubling the
            # result -- absorbed by the exp scale)
            QT = [
                [const.tile([P, 512], BF16, name=f"qt{mi}_{ni}") for ni in range(NQ)]
                for mi in range(2)
            ]
            QT8 = [
                [const.tile([P, 1, 512], FP8, name=f"qt8{mi}_{ni}") for ni in range(NQ)]
                for mi in range(2)
            ]
            KT8 = [
                [const.tile([P, 1, 512], FP8, name=f"kt8{mi}_{ni}") for ni in range(NQ)]
                for mi in range(2)
            ]
            vt = {}
            for pair in range(2):
                for i in range(16):
                    vt[(pair, i)] = const.tile([P, 2, 65], BF16, name=f"vt{pair}_{i}")
            attnT = {}  # (pair, m-pair) -> [P, 2, P]; [:, m%2, :] = attn^T(m)
            for pair in range(2):
                for mp in range(8):
                    attnT[(pair, mp)] = const.tile([P, 2, P], BF16, name=f"at{pair}_{mp}")

            # ones column of every V' tile (Pool, pair-0 first); written once
            for key in vt:
                nc.gpsimd.memset(vt[key][:, :, 64:65], 1.0)

            # ---- DMAs, ordered so the first Q/K sweeps can start early.
            # x arrives in 512-column (q-chunk) slabs, k-major inside.
            xq = [[None, None] for _ in range(NQ)]  # [qc][lohi] -> [P, 4, 512]
            x8q = [None] * NQ  # [qc] -> [P, 8, 512] fp8
            wq_big = const.tile([P, NKD, HD], BF16, name="wqb")
            wk8_big = const.tile([P, NKD, HD], FP8, name="wkb")

            def dma_x16(qc):
                cs = slice(qc * 512, (qc + 1) * 512)
                for lohi in range(2):
                    t = xtp.tile([P, 4, 512], BF16, name="xq")
                    nc.sync.dma_start(
                        out=t[:],
                        in_=xT[lohi * 512 : (lohi + 1) * 512, cs].rearrange(
                            "(k p) s -> p k s", p=P
                        ),
                    )
                    xq[qc][lohi] = t

            def dma_x8(qc):
                cs = slice(qc * 512, (qc + 1) * 512)
                t8 = x8p.tile([P, 8, 512], FP8, name="x8q")
                nc.sync.dma_start(
                    out=t8[:], in_=x8[:, cs].rearrange("(k p) s -> p k s", p=P)
                )
                x8q[qc] = t8

            def quant_x8(qc):
                # derive the fp8 copy on-chip (Pool) to shorten the serialized
                # input DMA stream; only used for the later q-chunks
                t8 = x8p.tile([P, 8, 512], FP8, name="x8q")
                for lohi in range(2):
                    nc.gpsimd.tensor_copy(
                        t8[:, 4 * lohi : 4 * lohi + 4, :], xq[qc][lohi][:]
                    )
                x8q[qc] = t8

            nc.sync.dma_start(out=wq_big[:], in_=Wq[:, :, :])
            dma_x16(0)
            nc.sync.dma_start(out=bq_sb[:], in_=bqt[:, :])
            nc.sync.dma_start(out=bk_sb[:], in_=bkt[:, :])
            nc.sync.dma_start(out=wk8_big[:], in_=Wk8[:, :, :])
            dma_x8(0)
            for qc in range(1, NQ):
                dma_x16(qc)
                dma_x8(qc)
            for p in range(2):
                nc.sync.dma_start(
                    out=w0_sb[p][:],
                    in_=W0[p * P : (p + 1) * P, :],
                )

            # ---- sweep emitters (as drip-able item lists)
            def q_sweep_items(ni, mi):
                ps = mmp.tile([P, 512], F32, name="ps")

                def mk(k):
                    def go():
                        nc.tensor.matmul(
                            ps[:],
                            lhsT=wq_big[:, k, mi * P : (mi + 1) * P],
                            rhs=xq[ni][k // 4][:, k % 4, :],
                            start=(k == 0),
                            stop=(k == NKD - 1),
                        )

                    return go

                def evict():
                    # two direct evictions: fp8 for scores (critical path),
                    # bf16 for the V' transposes
                    nc.vector.tensor_scalar_add(
                        QT8[mi][ni][:, 0, :], ps[:], bq_sb[:, mi : mi + 1]
                    )
                    nc.vector.tensor_scalar_add(
                        QT[mi][ni][:, :], ps[:], bq_sb[:, mi : mi + 1]
                    )

                return [(2, mk(k)) for k in range(NKD)] + [(1, evict)]

            def k_sweep_items(ni, mi):
                ps = mmp.tile([P, 512], F32, name="ps")

                def mk(kp):
                    def go():
                        nc.tensor.matmul(
                            ps[:],
                            lhsT=wk8_big[:, 2 * kp : 2 * kp + 2, mi * P : (mi + 1) * P],
                            rhs=x8q[ni][:, 2 * kp : 2 * kp + 2, :],
                            start=(kp == 0),
                            stop=(kp == 3),
                            perf_mode=DR,
                        )

                    return go

                def evict():
                    nc.vector.tensor_scalar(
                        out=KT8[mi][ni][:, 0, :],
                        in0=ps[:],
                        scalar1=1.0 / 128.0,
                        scalar2=bk_sb[:, mi : mi + 1],
                        op0=MUL,
                        op1=ADD,
                    )

                return [(2, mk(kp)) for kp in range(4)] + [(1, evict)]

            def vT_items(pair, ni):
                """one batched DMA transpose per QT tile covers 4 V' chunks"""
                state = {}

                def tp_go():
                    state["tp"] = tstg.tile([P, 4, P], BF16, name="ts")
                    nc.sync.dma_start_transpose(out=state["tp"][:], in_=QT[pair][ni][:, :])

                def cp(c):
                    def go():
                        v = vt[(pair, 4 * ni + c)]
                        src_ap = state["tp"][:, c, :].rearrange("p (h d) -> p h d", h=2)
                        nc.gpsimd.tensor_copy(v[:, :, 0:64], src_ap)

                    return go

                return [(2, tp_go)] + [(1, cp(c)) for c in range(4)]

            # ---- drip queue: background emit-thunks (sweeps, V'T, outproj)
            # items may carry a min step number (global exp-step counter) so
            # work that waits on a fresh DMA-transpose isn't popped while its
            # input is still in flight (it would stall the in-order PE queue)
            bg = []
            stepno = [0]

            def drip(budget):
                i2 = 0
                while i2 < len(bg) and budget > 0:
                    item = bg[i2]
                    if len(item) == 3 and item[2] > stepno[0]:
                        i2 += 1
                        continue
                    bg.pop(i2)
                    item[1]()
                    budget -= item[0]

            def emit_outproj(m, endgame=False):
                # endgame (post-last-exp): evictions alternate ACT/DVE (ACT is
                # idle by then) and the out DMA goes per-half to start earlier
                state = {}

                def half(n):
                    ps = mmp.tile([P, 512], F32, name="ps")
                    for p_ in range(2):
                        nc.tensor.matmul(
                            ps[:],
                            lhsT=attnT[(p_, m // 2)][:, m % 2, :],
                            rhs=w0_sb[p_][:, n * 512 : (n + 1) * 512],
                            start=(p_ == 0),
                            stop=(p_ == 1),
                        )
                    dst = state["ot"][:, n * 512 : (n + 1) * 512]
                    if endgame and n == 0:
                        nc.scalar.copy(dst, ps[:])
                    else:
                        nc.vector.tensor_copy(dst, ps[:])

                def go0():
                    state["ot"] = ostp.tile([P, D], BF16, name="ot")
                    half(0)
                    if endgame:
                        nc.sync.dma_start(
                            out=out[m * P : (m + 1) * P, 0:512],
                            in_=state["ot"][:, 0:512],
                        )

                def go1():
                    half(1)
                    if endgame:
                        nc.sync.dma_start(
                            out=out[m * P : (m + 1) * P, 512:1024],
                            in_=state["ot"][:, 512:1024],
                        )
                    else:
                        nc.sync.dma_start(
                            out=out[m * P : (m + 1) * P, :], in_=state["ot"][:]
                        )

                return [(2, go0), (2, go1)]

            # ---- attention
            def S_mm(pair, j, i):
                """score matmuls for tile (j, i): S^T doubled via the stride-0
                second k-tile; the 2x and 1/sqrt(DK) sit in the exp scale.
                A lands at [off:512], B at [512:512+w] so one exp covers both."""
                off = max(0, i * P - j * 512)
                w = 512 - off
                kc = slice((i % 4) * P, (i % 4 + 1) * P)
                sAB = sps.tile([P, 1024], F32, name="sab")
                qs = slice(off, 512)
                for h in range(2):
                    hs = slice(h * 64, h * 64 + 64)
                    dst = sAB[:, off:512] if h == 0 else sAB[:, 512 : 512 + w]
                    nc.tensor.matmul(
                        dst,
                        lhsT=KT8[pair][i // 4][hs, :, kc].broadcast_to([64, 2, P]),
                        rhs=QT8[pair][j][hs, :, qs].broadcast_to([64, 2, w]),
                        perf_mode=DR,
                    )
                return sAB

            def S_exp(pair, j, i, sAB):
                """one exp (+ causal masks) for tile (j, i); returns probs."""
                off = max(0, i * P - j * 512)
                w = 512 - off
                pAB = ppool.tile([P, 1024], BF16, name="pab")
                nc.scalar.activation(
                    out=pAB[:, off : 512 + w],
                    in_=sAB[:, off : 512 + w],
                    func=EXP,
                    scale=0.0625,
                )
                if i >= 4 * j:  # diagonal tile: mask the leading 128-col block
                    nc.gpsimd.tensor_mul(
                        pAB[:, off : off + P], pAB[:, off : off + P], tri[:]
                    )
                    nc.gpsimd.tensor_mul(
                        pAB[:, 512 : 512 + P], pAB[:, 512 : 512 + P], tri[:]
                    )
                return pAB

            def av_mm(pair, att, s, m, j, i, pAB):
                # each att bank holds one accumulation GROUP spanning both m
                # slots: start only zeroes once (it clears the whole 2KB zero
                # region), stop only on the very last write to the bank
                cm = (m - 4 * j) * P
                off = max(0, i * P - j * 512)
                last = None
                for h in range(2):
                    lo = cm if h == 0 else 512 + cm - off
                    base = (2 * s + h) * 65
                    last = nc.tensor.matmul(
                        att[:, base : base + 65],
                        lhsT=pAB[:, lo : lo + P],
                        rhs=vt[(pair, i)][:, h, :],
                        start=(i == 0 and s == 0 and h == 0),
                        stop=(i == m and s == 1 and h == 1),
                    )
                return last

            def normalize(pair, att, s, m, an, dep=None, endg=False):
                rc = rcp.tile([P, 2], F32, name="rc")
                for h in range(2):
                    base = (2 * s + h) * 65
                    r = nc.vector.reciprocal(
                        rc[:, h : h + 1], att[:, base + 64 : base + 65]
                    )
                    if dep is not None and h == 0:
                        # slot-0 values are final, but the bank's accumulation
                        # group only closes at the slot-1 stop matmul; DVE is
                        # in-order so one dep covers the whole normalize
                        add_dep_helper(r.ins, dep.ins, sync=True,
                                       reason="att group close")
                    nc.vector.tensor_scalar(
                        out=an[:, 128 * s + h * 64 : 128 * s + (h + 1) * 64],
                        in0=att[:, base : base + 64],
                        scalar1=rc[:, h : h + 1],
                        scalar2=None,
                        op0=MUL,
                    )

            def av_step(j, ms, att, ip, probs, op, pair=None):
                raise NotImplementedError

            def emit_pair(pair, jorder, budget):
                def av_step(j, ms, att, ip, probs, op, endg=False):
                    for m in ms:
                        if m < ip:
                            continue
                        t, s = att[m]
                        stop = av_mm(pair, t, s, m, j, ip, probs[ip])
                        if ip == m and s == 1:
                            # group closed: normalize both slots of this bank,
                            # then one transpose covers the m-pair. In the
                            # endgame the PE+DVE path beats the ~2.4us DMA
                            # XBAR transpose launch latency (PSUM is free).
                            an = anp.tile([P, 256], BF16, name="an")
                            normalize(pair, t, 0, m - 1, an, dep=stop)
                            normalize(pair, t, 1, m, an)
                            if endg:
                                for mm2 in range(2):
                                    tp = sps.tile([P, P], BF16, name="sab")
                                    nc.tensor.transpose(
                                        tp[:],
                                        an[:, 128 * mm2 : 128 * mm2 + 128],
                                        identity[:],
                                    )
                                    nc.vector.tensor_copy(
                                        attnT[(pair, m // 2)][:, mm2, :], tp[:]
                                    )
                            else:
                                nc.sync.dma_start_transpose(
                                    out=attnT[(pair, m // 2)][:], in_=an[:]
                                )
                            if pair == 1:
                                op(m - 1)
                                op(m)

                # scores run one step ahead of exps (lead-1) so the exp's
                # input semaphore has fired long before ACT gets there
                seq = [(j, i) for j in jorder for i in range(4 * j + 4)]
                sq = {}
                sq[seq[0]] = S_mm(pair, *seq[0])
                idx = 0
                for j in jorder:
                    last = pair == 1 and j == jorder[-1]

                    def op(m, last=last):
                        if last:
                            for _, it in emit_outproj(m, endgame=True):
                                it()
                        else:
                            bg.extend(
                                (c, t, stepno[0] + 3) for c, t in emit_outproj(m)
                            )

                    nsteps = 4 * j + 4
                    probs = {}
                    ms = list(range(4 * j, 4 * j + 4))
                    att = {}  # m -> (tile, slot)
                    pend = []  # i's whose AV is not yet emitted
                    for i in range(nsteps):
                        if idx + 1 < len(seq):
                            sq[seq[idx + 1]] = S_mm(pair, *seq[idx + 1])
                        probs[i] = S_exp(pair, j, i, sq.pop((j, i)))
                        idx += 1
                        stepno[0] += 1
                        pend.append(i)
                        # scale the dripped background work to this step's exp
                        # length so the PE never outruns ACT on short tiles
                        w = 512 - max(0, i * P - j * 512)
                        drip(max(2, budget * (512 + w) // 1024))
                        if i == 0:
                            lo = aps.tile([P, 260], F32, name="att")
                            hi = aps.tile([P, 260], F32, name="att")
                            for s, m in enumerate(ms):
                                att[m] = (lo, s) if s < 2 else (hi, s - 2)
                        if i >= 3:
                            ip = pend.pop(0)
                            av_step(j, ms, att, ip, probs, op, endg=last)
                    while pend:
                        ip = pend.pop(0)
                        av_step(j, ms, att, ip, probs, op, endg=last)

            # ---- schedule
            # upfront: first Q/K sweeps + first V' transposes (gate the first
            # score tile), everything else drips
            qs_up = q_sweep_items(0, 0)
            ks_up = k_sweep_items(0, 0)
            for _, it in qs_up[0:4]:
                it()
            for _, it in ks_up[:-1]:
                it()
            for _, it in qs_up[4:8]:
                it()
            ks_up[-1][1]()  # K eviction first (its data lands earlier)
            qs_up[-1][1]()  # then both Q evictions

            for _, it in vT_items(0, 0):
                it()

            order = []
            for ni in (1, 2, 3):
                order += q_sweep_items(ni, 0) + k_sweep_items(ni, 0)
                order += vT_items(0, ni)
            for ni in range(4):
                order += q_sweep_items(ni, 1) + k_sweep_items(ni, 1)
                order += vT_items(1, ni)
            bg.extend(order)

            emit_pair(0, (0, 1, 2, 3), budget=7)
            emit_pair(1, (0, 1, 2, 3), budget=5)
            while bg:
                drip(6)

    nc.compile()
    return nc


def make_in_maps(pos_encode_toks, Wq, bq, Wk, bk, W0, b0):
    x = np.asarray(pos_encode_toks, dtype=np.float32)
    Wq = np.asarray(Wq, dtype=np.float32)
    bq = np.asarray(bq, dtype=np.float32)
    Wk = np.asarray(Wk, dtype=np.float32)
    bk = np.asarray(bk, dtype=np.float32)
    W0 = np.asarray(W0, dtype=np.float32)
    in_maps = []
    for core in range(8):
        b, g = divmod(core, 4)
        hs = slice(g * HD, (g + 1) * HD)
        xt = np.ascontiguousarray(x[b].T)
        in_maps.append(
            {
                "xT": xt.astype(ml_dtypes.bfloat16),
                "x8": xt.astype(ml_dtypes.float8_e4m3),
                "Wq": np.ascontiguousarray(
                    Wq[:, hs].reshape(8, P, HD).transpose(1, 0, 2)
                ).astype(ml_dtypes.bfloat16),
                "Wk8": np.ascontiguousarray(
                    (Wk[:, hs] * 128.0).reshape(8, P, HD).transpose(1, 0, 2)
                ).astype(ml_dtypes.float8_e4m3),
                "bqt": np.ascontiguousarray(bq[hs].reshape(2, P).T),
                "bkt": np.ascontiguousarray(bk[hs].reshape(2, P).T),
                "W0": np.ascontiguousarray(W0[hs, :]).astype(ml_dtypes.bfloat16),
            }
        )
    return in_maps


def assemble(results, b0):
    out = np.zeros((2, S, D), dtype=np.float32)
    for core in range(8):
        b = core // 4
        out[b] += results[core]["out"].astype(np.float32)
    out += np.asarray(b0, dtype=np.float32)
    return out


def kernel(pos_encode_toks, Wq, bq, Wk, bk, W0, b0):
    from concourse.bass_utils import run_bass_kernel_spmd

    global _CACHED_NC
    if _CACHED_NC is None:
        _CACHED_NC = build_nc()
    in_maps = make_in_maps(pos_encode_toks, Wq, bq, Wk, bk, W0, b0)
    res = run_bass_kernel_spmd(_CACHED_NC, in_maps, core_ids=list(range(8)))
    return assemble(res.results, b0)


# revision 10
# speedup vs baseline: 1.0297x; 1.0045x over previous
"""Multi-head causal self-attention (V=Q variant) on 8 Trainium2 cores, v2.

Sharding: batch (2) x head-group (4 groups of 4 heads). Each core computes
full-sequence attention for its 4 heads (2 head-pairs) of one batch element
plus its slice of the output projection; the host sums 4 partials per batch
and adds b0.

v2 redesign vs v1 (145.4us -> 108.8us in the cost model):
- K projection and scores run in fp8e4m3 DoubleRow (0.5 cycles/row in the
  model). Scores keep a 64-deep contraction at the DoubleRow rate via a
  stride-0 broadcast second k-tile (doubling the result); Wk is
  host-prescaled by 128 to keep fp8 normals; the 1/128 sits in the K
  eviction and the 2x and 1/sqrt(DK) in the exp scale (0.0625).
- AV is computed reversed: att[q,65] = probs_chunk^T @ [V|1], filling all
  128 output partitions per pass (the forward form filled only 65). The
  softmax denominator still falls out of the ones column. Each att PSUM
  bank holds 4 AV slots under ONE accumulation group (start zeroes the
  whole 2KB zero region; only the first/last matmuls set start/stop).
  Normalization = DVE reciprocal + per-partition scalar multiply once the
  group closes; attn^T for the output projection is rebuilt with batched
  DMA XBAR transposes (as are the V' tiles from QT).
- exp is one [128,1024] activation per score tile: the B head's scores
  land at column 512 so diagonal tiles stay contiguous with A's [off:512].
  Scores run one step ahead of exps (lead-1) and AV three steps behind, so
  ACT (the bottleneck engine, ~77us busy) is fed with minimal stalls.
- GPSIMD cannot touch PSUM on real hardware: all PSUM evictions live on
  DVE (ACT helps in the post-exp endgame); Pool does the SBUF-only work
  (causal masks, V' ones columns, warmup memsets).
- PSUM: scores 2x[128,1024] double-buffered (4 banks) + att 2 banks +
  proj/outproj shared pool (2 banks) = all 8 banks.
"""

import ml_dtypes
import numpy as np

import concourse.bacc as bacc
import concourse.mybir as mybir
from concourse.tile import TileContext, add_dep_helper

P = 128
S = 2048
D = 1024
HD = 256
DK = 64
NQ = 4  # q blocks of 512
NKD = 8  # D chunks of 128
F32 = mybir.dt.float32
BF16 = mybir.dt.bfloat16
FP8 = mybir.dt.float8e4
EXP = mybir.ActivationFunctionType.Exp
DR = mybir.MatmulPerfMode.DoubleRow
MUL = mybir.AluOpType.mult
ADD = mybir.AluOpType.add

_CACHED_NC = None


def build_nc():
    nc = bacc.Bacc("TRN2", target_bir_lowering=False, debug=False, num_devices=8)
    xT = nc.declare_dram_parameter("xT", [D, S], BF16, isOutput=False)
    x8 = nc.declare_dram_parameter("x8", [D, S], FP8, isOutput=False)
    Wq = nc.declare_dram_parameter("Wq", [P, NKD, HD], BF16, isOutput=False)
    Wk8 = nc.declare_dram_parameter("Wk8", [P, NKD, HD], FP8, isOutput=False)
    bqt = nc.declare_dram_parameter("bqt", [P, 2], F32, isOutput=False)
    bkt = nc.declare_dram_parameter("bkt", [P, 2], F32, isOutput=False)
    W0 = nc.declare_dram_parameter("W0", [HD, D], BF16, isOutput=False)
    out = nc.declare_dram_parameter("out", [S, D], BF16, isOutput=True)

    with TileContext(nc) as tc:
        with (
            tc.tile_pool(name="const", bufs=1) as const,
            tc.tile_pool(name="xt", bufs=8) as xtp,
            tc.tile_pool(name="x8p", bufs=4) as x8p,
            tc.tile_pool(name="pp", bufs=24) as ppool,
            tc.tile_pool(name="an", bufs=6) as anp,
            tc.tile_pool(name="tst", bufs=4) as tstg,
            tc.tile_pool(name="rc", bufs=6) as rcp,
            tc.tile_pool(name="ost", bufs=6) as ostp,
            tc.tile_pool(name="sps", bufs=2, space="PSUM") as sps,
            tc.tile_pool(name="aps", bufs=2, space="PSUM") as aps,
            tc.tile_pool(name="mm", bufs=2, space="PSUM") as mmp,
        ):
            # ---- PE p-state warmup first: dummy matmuls keep the PE busy
            # through the initial DMA wait so real matmuls start at full clock
            wa = const.tile([P, 256], BF16, name="wa")
            nc.gpsimd.memset(wa[:], 0.0)
            for _ in range(14):
                wps = mmp.tile([P, 256], F32, name="ps")
                nc.tensor.matmul(wps[:], lhsT=wa[:, 0:128], rhs=wa[:], start=True, stop=True)

            # ---- constants / static tiles
            identity = const.tile([P, P], BF16)
            nc.gpsimd.memset(identity[:], 0.0)
            nc.gpsimd.affine_select(
                out=identity[:],
                in_=identity[:],
                compare_op=mybir.AluOpType.not_equal,
                fill=1.0,
                base=0,
                pattern=[[-1, P]],
                channel_multiplier=1,
            )
            # triangular keep-mask [128,128]: 1.0 where q >= kv
            tri = const.tile([P, P], BF16, name="tri")
            nc.gpsimd.memset(tri[:], 1.0)
            nc.gpsimd.affine_select(
                out=tri[:],
                in_=tri[:],
                compare_op=mybir.AluOpType.is_ge,
                fill=0.0,
                base=0,
                pattern=[[1, P]],
                channel_multiplier=-1,
            )
            # ACT exp-table warmup while DMAs run
            warm = const.tile([P, 8], F32, name="warm")
            nc.gpsimd.memset(warm[:], 0.0)
            nc.scalar.activation(out=warm[:], in_=warm[:], func=EXP)

            bq_sb = const.tile([P, 2], F32)
            bk_sb = const.tile([P, 2], F32)
            w0_sb = [const.tile([P, D], BF16, name=f"w0_{p}") for p in range(2)]
            # projections: QT (bf16, feeds V' transposes), QT8/KT8 (fp8;
            # scores use a stride-0 broadcast second k-tile, doubling the
            # result -- absorbed by the exp scale)
            QT = [
                [const.tile([P, 512], BF16, name=f"qt{mi}_{ni}") for ni in range(NQ)]
                for mi in range(2)
            ]
            QT8 = [
                [const.tile([P, 1, 512], FP8, name=f"qt8{mi}_{ni}") for ni in range(NQ)]
                for mi in range(2)
            ]
            KT8 = [
                [const.tile([P, 1, 512], FP8, name=f"kt8{mi}_{ni}") for ni in range(NQ)]
                for mi in range(2)
            ]
            vt = {}
            for pair in range(2):
                for i in range(16):
                    vt[(pair, i)] = const.tile([P, 2, 65], BF16, name=f"vt{pair}_{i}")
            attnT = {}  # (pair, m-pair) -> [P, 2, P]; [:, m%2, :] = attn^T(m)
            for pair in range(2):
                for mp in range(8):
                    attnT[(pair, mp)] = const.tile([P, 2, P], BF16, name=f"at{pair}_{mp}")

            # ones column of every V' tile (Pool, pair-0 first); written once
            for key in vt:
                nc.gpsimd.memset(vt[key][:, :, 64:65], 1.0)

            # ---- DMAs, ordered so the first Q/K sweeps can start early.
            # x arrives in 512-column (q-chunk) slabs, k-major inside.
            xq = [[None, None] for _ in range(NQ)]  # [qc][lohi] -> [P, 4, 512]
            x8q = [None] * NQ  # [qc] -> [P, 8, 512] fp8
            wq_big = const.tile([P, NKD, HD], BF16, name="wqb")
            wk8_big = const.tile([P, NKD, HD], FP8, name="wkb")

            def dma_x16(qc):
                cs = slice(qc * 512, (qc + 1) * 512)
                for lohi in range(2):
                    t = xtp.tile([P, 4, 512], BF16, name="xq")
                    nc.sync.dma_start(
                        out=t[:],
                        in_=xT[lohi * 512 : (lohi + 1) * 512, cs].rearrange(
                            "(k p) s -> p k s", p=P
                        ),
                    )
                    xq[qc][lohi] = t

            def dma_x8(qc):
                cs = slice(qc * 512, (qc + 1) * 512)
                t8 = x8p.tile([P, 8, 512], FP8, name="x8q")
                nc.sync.dma_start(
                    out=t8[:], in_=x8[:, cs].rearrange("(k p) s -> p k s", p=P)
                )
                x8q[qc] = t8

            def quant_x8(qc):
                # derive the fp8 copy on-chip (Pool) to shorten the serialized
                # input DMA stream; only used for the later q-chunks
                t8 = x8p.tile([P, 8, 512], FP8, name="x8q")
                for lohi in range(2):
                    nc.gpsimd.tensor_copy(
                        t8[:, 4 * lohi : 4 * lohi + 4, :], xq[qc][lohi][:]
                    )
                x8q[qc] = t8

            nc.sync.dma_start(out=wq_big[:], in_=Wq[:, :, :])
            dma_x16(0)
            nc.sync.dma_start(out=bq_sb[:], in_=bqt[:, :])
            nc.sync.dma_start(out=bk_sb[:], in_=bkt[:, :])
            nc.sync.dma_start(out=wk8_big[:], in_=Wk8[:, :, :])
            dma_x8(0)
            for qc in range(1, NQ):
                dma_x16(qc)
                dma_x8(qc)
            for p in range(2):
                nc.sync.dma_start(
                    out=w0_sb[p][:],
                    in_=W0[p * P : (p + 1) * P, :],
                )

            # ---- sweep emitters (as drip-able item lists)
            def q_sweep_items(ni, mi):
                ps = mmp.tile([P, 512], F32, name="ps")

                def mk(k):
                    def go():
                        nc.tensor.matmul(
                            ps[:],
                            lhsT=wq_big[:, k, mi * P : (mi + 1) * P],
                            rhs=xq[ni][k // 4][:, k % 4, :],
                            start=(k == 0),
                            stop=(k == NKD - 1),
                        )

                    return go

                def evict():
                    # two direct evictions: fp8 for scores (critical path),
                    # bf16 for the V' transposes
                    nc.vector.tensor_scalar_add(
                        QT8[mi][ni][:, 0, :], ps[:], bq_sb[:, mi : mi + 1]
                    )
                    nc.vector.tensor_scalar_add(
                        QT[mi][ni][:, :], ps[:], bq_sb[:, mi : mi + 1]
                    )

                return [(2, mk(k)) for k in range(NKD)] + [(1, evict)]

            def k_sweep_items(ni, mi):
                ps = mmp.tile([P, 512], F32, name="ps")

                def mk(kp):
                    def go():
                        nc.tensor.matmul(
                            ps[:],
                            lhsT=wk8_big[:, 2 * kp : 2 * kp + 2, mi * P : (mi + 1) * P],
                            rhs=x8q[ni][:, 2 * kp : 2 * kp + 2, :],
                            start=(kp == 0),
                            stop=(kp == 3),
                            perf_mode=DR,
                        )

                    return go

                def evict():
                    nc.vector.tensor_scalar(
                        out=KT8[mi][ni][:, 0, :],
                        in0=ps[:],
                        scalar1=1.0 / 128.0,
                        scalar2=bk_sb[:, mi : mi + 1],
                        op0=MUL,
                        op1=ADD,
                    )

                return [(2, mk(kp)) for kp in range(4)] + [(1, evict)]

            def vT_items(pair, ni):
                """one batched DMA transpose per QT tile covers 4 V' chunks"""
                state = {}

                def tp_go():
                    state["tp"] = tstg.tile([P, 4, P], BF16, name="ts")
                    nc.sync.dma_start_transpose(out=state["tp"][:], in_=QT[pair][ni][:, :])

                def cp(c):
                    def go():
                        v = vt[(pair, 4 * ni + c)]
                        src_ap = state["tp"][:, c, :].rearrange("p (h d) -> p h d", h=2)
                        nc.gpsimd.tensor_copy(v[:, :, 0:64], src_ap)

                    return go

                return [(2, tp_go)] + [(1, cp(c)) for c in range(4)]

            # ---- drip queue: background emit-thunks (sweeps, V'T, outproj)
            # items may carry a min step number (global exp-step counter) so
            # work that waits on a fresh DMA-transpose isn't popped while its
            # input is still in flight (it would stall the in-order PE queue)
            bg = []
            stepno = [0]

            def drip(budget):
                i2 = 0
                while i2 < len(bg) and budget > 0:
                    item = bg[i2]
                    if len(item) == 3 and item[2] > stepno[0]:
                        i2 += 1
                        continue
                    bg.pop(i2)
                    item[1]()
                    budget -= item[0]

            def emit_outproj(m, endgame=False):
                # endgame (post-last-exp): evictions alternate ACT/DVE (ACT is
                # idle by then) and the out DMA goes per-half to start earlier
                state = {}

                def half(n):
                    ps = mmp.tile([P, 512], F32, name="ps")
                    for p_ in range(2):
                        nc.tensor.matmul(
                            ps[:],
                            lhsT=attnT[(p_, m // 2)][:, m % 2, :],
                            rhs=w0_sb[p_][:, n * 512 : (n + 1) * 512],
                            start=(p_ == 0),
                            stop=(p_ == 1),
                        )
                    dst = state["ot"][:, n * 512 : (n + 1) * 512]
                    if endgame and n == 0:
                        nc.scalar.copy(dst, ps[:])
                    else:
                        nc.vector.tensor_copy(dst, ps[:])

                def go0():
                    state["ot"] = ostp.tile([P, D], BF16, name="ot")
                    half(0)
                    if endgame:
                        nc.sync.dma_start(
                            out=out[m * P : (m + 1) * P, 0:512],
                            in_=state["ot"][:, 0:512],
                        )

                def go1():
                    half(1)
                    if endgame:
                        nc.sync.dma_start(
                            out=out[m * P : (m + 1) * P, 512:1024],
                            in_=state["ot"][:, 512:1024],
                        )
                    else:
                        nc.sync.dma_start(
                            out=out[m * P : (m + 1) * P, :], in_=state["ot"][:]
                        )

                return [(2, go0), (2, go1)]

            # ---- attention
            def S_mm(pair, j, i):
                """score matmuls for tile (j, i): S^T doubled via the stride-0
                second k-tile; the 2x and 1/sqrt(DK) sit in the exp scale.
                A lands at [off:512], B at [512:512+w] so one exp covers both."""
                off = max(0, i * P - j * 512)
                w = 512 - off
                kc = slice((i % 4) * P, (i % 4 + 1) * P)
                sAB = sps.tile([P, 1024], F32, name="sab")
                qs = slice(off, 512)
                for h in range(2):
                    hs = slice(h * 64, h * 64 + 64)
                    dst = sAB[:, off:512] if h == 0 else sAB[:, 512 : 512 + w]
                    nc.tensor.matmul(
                        dst,
                        lhsT=KT8[pair][i // 4][hs, :, kc].broadcast_to([64, 2, P]),
                        rhs=QT8[pair][j][hs, :, qs].broadcast_to([64, 2, w]),
                        perf_mode=DR,
                    )
                return sAB

            def S_exp(pair, j, i, sAB):
                """one exp (+ causal masks) for tile (j, i); returns probs."""
                off = max(0, i * P - j * 512)
                w = 512 - off
                pAB = ppool.tile([P, 1024], BF16, name="pab")
                nc.scalar.activation(
                    out=pAB[:, off : 512 + w],
                    in_=sAB[:, off : 512 + w],
                    func=EXP,
                    scale=0.0625,
                )
                if i >= 4 * j:  # diagonal tile: mask the leading 128-col block
                    nc.gpsimd.tensor_mul(
                        pAB[:, off : off + P], pAB[:, off : off + P], tri[:]
                    )
                    nc.gpsimd.tensor_mul(
                        pAB[:, 512 : 512 + P], pAB[:, 512 : 512 + P], tri[:]
                    )
                return pAB

            def av_mm(pair, att, s, m, j, i, pAB):
                # each att bank holds one accumulation GROUP spanning both m
                # slots: start only zeroes once (it clears the whole 2KB zero
                # region), stop only on the very last write to the bank
                cm = (m - 4 * j) * P
                off = max(0, i * P - j * 512)
                last = None
                for h in range(2):
                    lo = cm if h == 0 else 512 + cm - off
                    base = (2 * s + h) * 65
                    last = nc.tensor.matmul(
                        att[:, base : base + 65],
                        lhsT=pAB[:, lo : lo + P],
                        rhs=vt[(pair, i)][:, h, :],
                        start=(i == 0 and s == 0 and h == 0),
                        stop=(i == m and s == 1 and h == 1),
                    )
                return last

            def normalize(pair, att, s, m, an, dep=None, endg=False):
                rc = rcp.tile([P, 2], F32, name="rc")
                for h in range(2):
                    base = (2 * s + h) * 65
                    r = nc.vector.reciprocal(
                        rc[:, h : h + 1], att[:, base + 64 : base + 65]
                    )
                    if dep is not None and h == 0:
                        # slot-0 values are final, but the bank's accumulation
                        # group only closes at the slot-1 stop matmul; DVE is
                        # in-order so one dep covers the whole normalize
                        add_dep_helper(r.ins, dep.ins, sync=True,
                                       reason="att group close")
                    nc.vector.tensor_scalar(
                        out=an[:, 128 * s + h * 64 : 128 * s + (h + 1) * 64],
                        in0=att[:, base : base + 64],
                        scalar1=rc[:, h : h + 1],
                        scalar2=None,
                        op0=MUL,
                    )

            def av_step(j, ms, att, ip, probs, op, pair=None):
                raise NotImplementedError

            def emit_pair(pair, jorder, budget):
                def av_step(j, ms, att, ip, probs, op, endg=False):
                    for m in ms:
                        if m < ip:
                            continue
                        t, s = att[m]
                        stop = av_mm(pair, t, s, m, j, ip, probs[ip])
                        if ip == m and s == 1:
                            # group closed: normalize both slots of this bank,
                            # then one transpose covers the m-pair. In the
                            # endgame the PE+DVE path beats the ~2.4us DMA
                            # XBAR transpose launch latency (PSUM is free).
                            an = anp.tile([P, 256], BF16, name="an")
                            normalize(pair, t, 0, m - 1, an, dep=stop)
                            normalize(pair, t, 1, m, an)
                            if endg:
                                for mm2 in range(2):
                                    tp = sps.tile([P, P], BF16, name="sab")
                                    nc.tensor.transpose(
                                        tp[:],
                                        an[:, 128 * mm2 : 128 * mm2 + 128],
                                        identity[:],
                                    )
                                    nc.vector.tensor_copy(
                                        attnT[(pair, m // 2)][:, mm2, :], tp[:]
                                    )
                            else:
                                nc.sync.dma_start_transpose(
                                    out=attnT[(pair, m // 2)][:], in_=an[:]
                                )
                            if pair == 1:
                                op(m - 1)
                                op(m)

                # scores run one step ahead of exps (lead-1) so the exp's
                # input semaphore has fired long before ACT gets there
                seq = [(j, i) for j in jorder for i in range(4 * j + 4)]
                sq = {}
                sq[seq[0]] = S_mm(pair, *seq[0])
                idx = 0
                for j in jorder:
                    last = pair == 1 and j == jorder[-1]

                    def op(m, last=last):
                        if last:
                            for _, it in emit_outproj(m, endgame=True):
                                it()
                        else:
                            bg.extend(
                                (c, t, stepno[0] + 3) for c, t in emit_outproj(m)
                            )

                    nsteps = 4 * j + 4
                    probs = {}
                    ms = list(range(4 * j, 4 * j + 4))
                    att = {}  # m -> (tile, slot)
                    pend = []  # i's whose AV is not yet emitted
                    for i in range(nsteps):
                        if idx + 1 < len(seq):
                            sq[seq[idx + 1]] = S_mm(pair, *seq[idx + 1])
                        probs[i] = S_exp(pair, j, i, sq.pop((j, i)))
                        idx += 1
                        stepno[0] += 1
                        pend.append(i)
                        # scale the dripped background work to this step's exp
                        # length so the PE never outruns ACT on short tiles
                        w = 512 - max(0, i * P - j * 512)
                        drip(max(2, budget * (512 + w) // 1024))
                        if i == 0:
                            lo = aps.tile([P, 260], F32, name="att")
                            hi = aps.tile([P, 260], F32, name="att")
                            for s, m in enumerate(ms):
                                att[m] = (lo, s) if s < 2 else (hi, s - 2)
                        if i >= 3:
                            ip = pend.pop(0)
                            av_step(j, ms, att, ip, probs, op, endg=last)
                    while pend:
                        ip = pend.pop(0)
                        av_step(j, ms, att, ip, probs, op, endg=last)

            # ---- schedule
            # upfront: first Q/K sweeps + first V' transposes (gate the first
            # score tile), everything else drips
            qs_up = q_sweep_items(0, 0)
            ks_up = k_sweep_items(0, 0)
            for _, it in qs_up[0:4]:
                it()
            for _, it in ks_up[:-1]:
                it()
            for _, it in qs_up[4:8]:
                it()
            ks_up[-1][1]()  # K eviction first (its data lands earlier)
            qs_up[-1][1]()  # then both Q evictions

            for _, it in vT_items(0, 0):
                it()

            order = []
            for ni in (1, 2, 3):
                order += q_sweep_items(ni, 0) + k_sweep_items(ni, 0)
                order += vT_items(0, ni)
            for ni in range(4):
                order += q_sweep_items(ni, 1) + k_sweep_items(ni, 1)
                order += vT_items(1, ni)
            bg.extend(order)

            emit_pair(0, (0, 1, 2, 3), budget=7)
            emit_pair(1, (0, 1, 2, 3), budget=5)
            while bg:
                drip(6)

    nc.compile()
    return nc


def make_in_maps(pos_encode_toks, Wq, bq, Wk, bk, W0, b0):
    x = np.asarray(pos_encode_toks, dtype=np.float32)
    Wq = np.asarray(Wq, dtype=np.float32)
    bq = np.asarray(bq, dtype=np.float32)
    Wk = np.asarray(Wk, dtype=np.float32)
    bk = np.asarray(bk, dtype=np.float32)
    W0 = np.asarray(W0, dtype=np.float32)
    in_maps = []
    for core in range(8):
        b, g = divmod(core, 4)
        hs = slice(g * HD, (g + 1) * HD)
        xt = np.ascontiguousarray(x[b].T)
        in_maps.append(
            {
                "xT": xt.astype(ml_dtypes.bfloat16),
                "x8": xt.astype(ml_dtypes.float8_e4m3),
                "Wq": np.ascontiguousarray(
                    Wq[:, hs].reshape(8, P, HD).transpose(1, 0, 2)
                ).astype(ml_dtypes.bfloat16),
                "Wk8": np.ascontiguousarray(
                    (Wk[:, hs] * 128.0).reshape(8, P, HD).transpose(1, 0, 2)
                ).astype(ml_dtypes.float8_e4m3),
                "bqt": np.ascontiguousarray(bq[hs].reshape(2, P).T),
                "bkt": np.ascontiguousarray(bk[hs].reshape(2, P).T),
                "W0": np.ascontiguousarray(W0[hs, :]).astype(ml_dtypes.bfloat16),
            }
        )
    return in_maps


def assemble(results, b0):
    out = np.zeros((2, S, D), dtype=np.float32)
    for core in range(8):
        b = core // 4
        out[b] += results[core]["out"].astype(np.float32)
    out += np.asarray(b0, dtype=np.float32)
    return out


def kernel(pos_encode_toks, Wq, bq, Wk, bk, W0, b0):
    from concourse.bass_utils import run_bass_kernel_spmd

    global _CACHED_NC
    if _CACHED_NC is None:
        _CACHED_NC = build_nc()
    in_maps = make_in_maps(pos_encode_toks, Wq, bq, Wk, bk, W0, b0)
    res = run_bass_kernel_spmd(_CACHED_NC, in_maps, core_ids=list(range(8)))
    return assemble(res.results, b0)
